# revision 1
# baseline (speedup 1.0000x reference)
"""GQA int8-KV-cache decode attention on 8 NeuronCores (Bass/Tile).

Sharding: kv-head axis (8 kv heads -> 1 per core), per the tensor-parallel hint.

Design (v3):
- Cache values fit int8 (ints in [-127,127], exact in bf16). Host repacks each
  core's K shard TRANSPOSED to [d, s] and V shard chunk-swizzled to
  [s%128, chunk, d], both int8. Two batches are packed per DRAM row-block so
  the sweep needs only 8 large DMAs (~1.3MB each, 2.6KB/partition lines):
  4x less HBM traffic than the int32 source and few enough DMAs that the
  issue path does not throttle the 360GB/s stream.
- Batches are processed in descending chunk-count order so the last DMA to
  land carries the smallest batches (short pipeline tail).
- HW-measured: on-chip convert is the binding resource; spreading it across
  DVE/GpSimd/ACT is counterproductive (shared SBUF ports). Best: ALL bytes
  convert inside the DMA datapath (SWDGE cast-DMA int8->bf16, no engine
  cycles); engines only run the softmax.
- Scores: per 128-chunk matmul with the KT chunk as stationary weights (FWL)
  and the tiny q [d,4] as moving operand; both batches of a pair land in ONE
  PSUM bank -> softmax is 3 per-pair instructions (DVE mul k_scaler bcast,
  ACT exp, DVE mul v_scaler -> bf16) instead of per-chunk ops.
- PV: V chunk as stationary weights, pexp [s,4] moving; accumulates [d, 4]
  per batch in a single PSUM bank across chunks. PV for batch i is emitted
  after scores for batch i+1 to keep PE streaming.
- Masking is folded into host-prepared scalers: ks'=0, vs'=0 at positions
  s > input_pos. Masked positions then contribute exp(0)=1 to the softmax
  denominator, which the host subtracts exactly; the numerator gets 0.
- Softmax denominator: DVE free-dim reduce of exp per batch -> [128, 4]
  partials; partition-sum + normalize + output transpose on host (O(B*H*D)).
- Small tensors (q, scalers) ride the gpsimd SWDGE queue; outputs are merged
  into one [128, B, 8] tensor -> single store DMA.

Measured on HW (WRAP-delta wall clock, 8 cores): 163.5us with 3-engine
convert split -> 54.6us with CAST=1.0 + DVE-only convert (rel err 4.1e-3).
Baseline int32 cast-DMA kernel: ~146us (CoreSim estimate).

Env knobs: KERNEL_WRAP (on-device repeat loop for wall-clock timing),
KERNEL_SPLIT="dve_frac,gps_frac" engine-convert split, KERNEL_CAST
(cast-DMA fraction), KERNEL_BUFS, KERNEL_MODE (dmaonly/fastconv/nope
timing-isolation variants).
"""

import os

os.environ.setdefault("JAX_PLATFORMS", "cpu")

import math
import numpy as np

B, H, KVH, D, CACHE = 16, 32, 8, 128, 4096
NREP = H // KVH
NCORES = 8
CHUNK = 128
NCH_MAX = CACHE // CHUNK  # 32

WRAP = int(os.environ.get("KERNEL_WRAP", "1"))
MODE = os.environ.get("KERNEL_MODE", "full")
# int8->bf16 convert split over the pair tile: DVE fraction, Pool fraction
# (remainder goes to ACT)
SPLIT = os.environ.get("KERNEL_SPLIT", "1.0,0.0")
# fraction of each pair's columns loaded via SWDGE cast-DMA (int8 HBM ->
# bf16 SBUF in the DMA datapath, no engine cycles); rest is raw int8 +
# engine convert per SPLIT
CAST = float(os.environ.get("KERNEL_CAST", "1.0"))
BUFS = int(os.environ.get("KERNEL_BUFS", "3"))

_BUILD_CACHE = {}
LAST_RESULTS = None


def _rope(x, cos, sin):
    # x: [B, 1, Hx, D]; cos/sin: [B, 1, D//2]
    c = cos[:, :, None, :]
    s = sin[:, :, None, :]
    xe, xo = x[..., ::2], x[..., 1::2]
    re = xe * c - xo * s
    im = xe * s + xo * c
    return np.stack([re, im], axis=-1).reshape(x.shape).astype(np.float32)


def _order_pairs(ncs):
    # Processing order: small pair first (compute starts as early as
    # possible), descending middle (suffix engine-work shrinks faster than
    # the remaining DMA stream), smallest pair last (shortest tail).
    asc = sorted(range(B), key=lambda b: (ncs[b], b))
    order = asc[0:2] + asc[:3:-1] + asc[2:4]
    pairs = [(order[i], order[i + 1]) for i in range(0, B, 2)]
    return order, pairs


def _build_program(ncs):
    """ncs: tuple of per-batch chunk counts (same program for every core)."""
    from contextlib import ExitStack

    import concourse.bacc as bacc
    import concourse.tile as tile
    from concourse import mybir

    nc = bacc.Bacc()
    f32 = mybir.dt.float32
    bf16 = mybir.dt.bfloat16
    i8 = mybir.dt.int8

    _, pairs = _order_pairs(ncs)

    order, pairs = _order_pairs(ncs)
    # packed scaler columns: per pair pi, [ks_a | ks_b | vs_a | vs_b]
    poff = []
    off = 0
    for ba, bb in pairs:
        poff.append(off)
        off += 2 * (ncs[ba] + ncs[bb])
    scl_cols = off

    # Per pair (ba, bb): row = [KT(ba) | Vsw(ba) | KT(bb) | Vsw(bb)],
    # KT = [d, s] K-transposed, Vsw = [s%128, c, d] chunk-swizzled V.
    kv = nc.dram_tensor("kv", [B // 2, CHUNK, 4 * CACHE], i8, kind="ExternalInput")
    scl = nc.dram_tensor("scl", [CHUNK, scl_cols], f32, kind="ExternalInput")
    qt = nc.dram_tensor("qt", [CHUNK, B, NREP], bf16, kind="ExternalInput")
    od = nc.dram_tensor("od", [CHUNK, B, 2 * NREP], f32, kind="ExternalOutput")

    dve_f, gps_f = (float(x) for x in SPLIT.split(","))

    with tile.TileContext(nc) as tc:
        with ExitStack() as ctx:
            singles = ctx.enter_context(tc.tile_pool(name="singles", bufs=1))
            kvi_pool = ctx.enter_context(tc.tile_pool(name="kvi", bufs=BUFS + 2))
            kvb_pool = ctx.enter_context(tc.tile_pool(name="kvb", bufs=BUFS))
            sc_pool = ctx.enter_context(
                tc.tile_pool(name="sc", bufs=4, space="PSUM")
            )
            ov_pool = ctx.enter_context(
                tc.tile_pool(name="ov", bufs=1, space="PSUM")
            )
            sm_pool = ctx.enter_context(tc.tile_pool(name="sm", bufs=4))

            qt_all = singles.tile([CHUNK, B, NREP], bf16)
            scl_all = singles.tile([CHUNK, scl_cols], f32)
            od_sb = singles.tile([CHUNK, B, 2 * NREP], f32)

            def body():
                opv = ov_pool.tile([CHUNK, B * NREP], f32, tag="opv")

                pend = []  # deferred PV emission
                done_pv = [0]  # count of batches with PV emitted

                if MODE in ("dmaonly", "nope"):
                    nc.vector.memset(opv, 0.0)
                    nc.vector.memset(
                        od_sb.rearrange("p b r -> p (b r)"), 0.0
                    )

                def emit_pv(vtb, pexp, oi, nch):
                    for c in range(nch if MODE != "nope" else 0):
                        nc.tensor.matmul(
                            opv[:, oi * NREP : (oi + 1) * NREP],
                            lhsT=vtb[:, c, :],
                            rhs=pexp[:, c, :],
                            start=(c == 0),
                            stop=(c == nch - 1),
                        )
                    done_pv[0] += 1
                    if done_pv[0] == B - 2:
                        # store the first B-2 processed batches early,
                        # overlapping the tail of the pipeline
                        lo = B - 2
                        nc.scalar.copy(
                            od_sb[:, 0:lo, 0:NREP],
                            opv[:, 0 : lo * NREP].rearrange(
                                "p (b r) -> p b r", r=NREP
                            ),
                        )
                        nc.sync.dma_start(
                            out=od[:, 0:lo, :], in_=od_sb[:, 0:lo, :]
                        )

                for pi, (ba, bb) in enumerate(pairs):
                    na, nb = ncs[ba], ncs[bb]
                    Sa, Sb = na * CHUNK, nb * CHUNK
                    ncp = na + nb
                    W = 2 * Sa + 2 * Sb
                    # wc..W loaded via SWDGE cast-DMA straight into kvb;
                    # 0..wc loaded raw int8 and engine-converted
                    wc = W - (int(W * CAST) // CHUNK) * CHUNK
                    kvb = kvb_pool.tile([CHUNK, W], bf16, tag="kvb")
                    if wc > 0:
                        kvi = kvi_pool.tile([CHUNK, wc], i8, tag="kvi")
                        nc.sync.dma_start(out=kvi, in_=kv[pi, :, 0:wc])
                    if W > wc:
                        nc.gpsimd.dma_start(
                            out=kvb[:, wc:W], in_=kv[pi, :, wc:W]
                        )
                    if pi == 0:
                        # small tensors go on the ACT HWDGE queue, which is
                        # idle until the first pair's data lands
                        nc.scalar.dma_start(out=qt_all, in_=qt[:, :, :])
                        nc.scalar.dma_start(out=scl_all, in_=scl[:, :])

                    if MODE == "dmaonly":
                        continue

                    # int8 -> bf16, three contiguous slices across engines
                    n0 = (int(wc * dve_f) // CHUNK) * CHUNK
                    n1 = n0 + (int(wc * gps_f) // CHUNK) * CHUNK
                    if MODE == "fastconv":
                        # timing-isolation: convert only a sliver (results wrong)
                        nc.vector.tensor_copy(kvb[:, 0:CHUNK], kvi[:, 0:CHUNK])
                    else:
                        if n0 > 0:
                            nc.vector.tensor_copy(kvb[:, 0:n0], kvi[:, 0:n0])
                        if n1 > n0:
                            nc.gpsimd.tensor_copy(
                                kvb[:, n0:n1], kvi[:, n0:n1]
                            )
                        if wc > n1:
                            nc.scalar.copy(kvb[:, n1:wc], kvi[:, n1:wc])

                    kta = kvb[:, 0:Sa]
                    vta = kvb[:, Sa : 2 * Sa].rearrange("p (c d) -> p c d", d=D)
                    ktb = kvb[:, 2 * Sa : 2 * Sa + Sb]
                    vtb = kvb[:, 2 * Sa + Sb : W].rearrange(
                        "p (c d) -> p c d", d=D
                    )

                    # both batches' scores into one PSUM tile:
                    # cols [0:na] = batch a, [na:ncp] = batch b
                    scp = sc_pool.tile([CHUNK, ncp, NREP], f32, tag="scp")
                    scpf = scp.rearrange("p c r -> p (c r)")
                    if MODE == "nope":
                        # timing-isolation: skip all PE matmuls (results wrong)
                        nc.vector.memset(scpf[:, 0:1], 1.0)
                    for c in range(na if MODE != "nope" else 0):
                        nc.tensor.matmul(
                            scpf[:, c * NREP : (c + 1) * NREP],
                            lhsT=kta[:, c * CHUNK : (c + 1) * CHUNK],
                            rhs=qt_all[:, ba, :],
                            start=True,
                            stop=True,
                        )
                    for c in range(nb if MODE != "nope" else 0):
                        nc.tensor.matmul(
                            scpf[:, (na + c) * NREP : (na + c + 1) * NREP],
                            lhsT=ktb[:, c * CHUNK : (c + 1) * CHUNK],
                            rhs=qt_all[:, bb, :],
                            start=True,
                            stop=True,
                        )

                    # pair-level softmax: scl packed as [ks_a|ks_b|vs_a|vs_b]
                    po = poff[pi]
                    ksb = scl_all[:, po : po + ncp].unsqueeze(2).broadcast_to(
                        [CHUNK, ncp, NREP]
                    )
                    vsb = (
                        scl_all[:, po + ncp : po + 2 * ncp]
                        .unsqueeze(2)
                        .broadcast_to([CHUNK, ncp, NREP])
                    )
                    et = sm_pool.tile([CHUNK, ncp, NREP], f32, tag="et")
                    nc.vector.tensor_tensor(
                        out=et, in0=scp, in1=ksb, op=mybir.AluOpType.mult
                    )
                    nc.scalar.activation(
                        et.rearrange("p c r -> p (c r)"),
                        et.rearrange("p c r -> p (c r)"),
                        mybir.ActivationFunctionType.Exp,
                    )
                    pexp = sm_pool.tile([CHUNK, ncp, NREP], bf16, tag="pexp")
                    nc.vector.tensor_tensor(
                        out=pexp, in0=et, in1=vsb, op=mybir.AluOpType.mult
                    )
                    # denominator partials per batch: sum_c exp -> [128, 4]
                    nc.vector.reduce_sum(
                        od_sb[:, 2 * pi, NREP : 2 * NREP],
                        et[:, 0:na, :].transpose([0, 2, 1]),
                        axis=mybir.AxisListType.X,
                    )
                    nc.vector.reduce_sum(
                        od_sb[:, 2 * pi + 1, NREP : 2 * NREP],
                        et[:, na:ncp, :].transpose([0, 2, 1]),
                        axis=mybir.AxisListType.X,
                    )

                    pend.append((vta, pexp[:, 0:na, :], 2 * pi, na))
                    pend.append((vtb, pexp[:, na:ncp, :], 2 * pi + 1, nb))
                    while len(pend) > 2:
                        emit_pv(*pend.pop(0))

                while pend:
                    emit_pv(*pend.pop(0))

                nc.scalar.copy(
                    od_sb[:, B - 2 : B, 0:NREP],
                    opv[:, (B - 2) * NREP :].rearrange(
                        "p (b r) -> p b r", r=NREP
                    ),
                )

            if WRAP > 1:
                with tc.For_i(0, WRAP, 1):
                    body()
            else:
                body()

            nc.sync.dma_start(
                out=od[:, B - 2 : B, :], in_=od_sb[:, B - 2 : B, :]
            )

    nc.compile()
    return nc


def _host_prep(
    xq, xk, xv, freqs_cos, freqs_sin, k_scaler, v_scaler, cache_k, cache_v, input_pos
):
    """Returns (in_maps, ncs, n_masked) for all cores."""
    import ml_dtypes

    bf16 = ml_dtypes.bfloat16
    pos = input_pos.astype(np.int64)
    bidx = np.arange(B)

    q = _rope(xq, freqs_cos, freqs_sin)[:, 0]  # [B, H, D]
    k = _rope(xk, freqs_cos, freqs_sin)[:, 0]  # [B, KVH, D]
    v_new = xv[:, 0]  # [B, KVH, D]
    k_s = (np.max(np.abs(k), axis=-1, keepdims=True) / np.float32(127.0)).astype(
        np.float32
    ) + np.float32(1e-8)
    v_s = (np.max(np.abs(v_new), axis=-1, keepdims=True) / np.float32(127.0)).astype(
        np.float32
    ) + np.float32(1e-8)
    k_q = np.clip(np.round(k / k_s), -127, 127).astype(np.int8)
    v_q = np.clip(np.round(v_new / v_s), -127, 127).astype(np.int8)

    ncs = tuple(int(p) // CHUNK + 1 for p in pos)
    n_masked = np.array([ncs[b] * CHUNK - (int(pos[b]) + 1) for b in range(B)],
                        np.float32)
    order, pairs = _order_pairs(ncs)
    scl_cols = 2 * sum(ncs)
    del order  # scl/kv are packed per pair; od is indexed by processing order

    inv_sqrt_d = np.float32(1.0 / math.sqrt(D))
    smask = np.arange(CACHE, dtype=np.int64)[None, :] > pos[:, None]  # [B, CACHE]

    def chunk_layout(a):  # [B, CACHE] -> [128, B, 32] with s = c*128 + p
        return np.ascontiguousarray(
            a.reshape(B, NCH_MAX, CHUNK).transpose(2, 0, 1)
        )

    in_maps = []
    for m in range(NCORES):
        k8 = cache_k[:, m].astype(np.int8)  # [B, CACHE, D]
        v8 = cache_v[:, m].astype(np.int8)
        k8[bidx, pos, :] = k_q[:, m]
        v8[bidx, pos, :] = v_q[:, m]

        kv_m = np.zeros((B // 2, CHUNK, 4 * CACHE), np.int8)
        for pi, (ba, bb) in enumerate(pairs):
            off = 0
            for b in (ba, bb):
                S = ncs[b] * CHUNK
                kv_m[pi, :, off : off + S] = k8[b, 0:S, :].T
                kv_m[pi, :, off + S : off + 2 * S] = (
                    v8[b, 0:S, :].reshape(ncs[b], CHUNK, D).transpose(1, 0, 2)
                    .reshape(CHUNK, S)
                )
                off += 2 * S

        ks_m = k_scaler[:, m].copy()  # [B, CACHE]
        vs_m = v_scaler[:, m].copy()
        ks_m[bidx, pos] = k_s[:, m, 0]
        vs_m[bidx, pos] = v_s[:, m, 0]
        ks_m *= inv_sqrt_d
        ks_m[smask] = 0.0
        vs_m[smask] = 0.0
        ksl = chunk_layout(ks_m)  # [128, B, 32]
        vsl = chunk_layout(vs_m)
        scl_m = np.zeros((CHUNK, scl_cols), np.float32)
        off = 0
        for ba, bb in pairs:
            na, nb = ncs[ba], ncs[bb]
            scl_m[:, off : off + na] = ksl[:, ba, 0:na]
            scl_m[:, off + na : off + na + nb] = ksl[:, bb, 0:nb]
            scl_m[:, off + na + nb : off + 2 * na + nb] = vsl[:, ba, 0:na]
            scl_m[:, off + 2 * na + nb : off + 2 * (na + nb)] = vsl[:, bb, 0:nb]
            off += 2 * (na + nb)

        qt_m = np.ascontiguousarray(
            q[:, m * NREP : (m + 1) * NREP, :].transpose(2, 0, 1)
        ).astype(bf16)  # [D, B, NREP]

        in_maps.append(dict(kv=kv_m, scl=np.ascontiguousarray(scl_m), qt=qt_m))
    return in_maps, ncs, n_masked


def _postprocess(results, ncs, n_masked):
    order, _ = _order_pairs(ncs)
    inv = np.argsort(np.array(order))  # processing index of original batch b
    out = np.zeros((B, H, 1, D), np.float32)
    for m in range(NCORES):
        od_m = results[m]["od"][:, inv, :]  # unpermute -> original batch order
        o_m = od_m[:, :, 0:NREP]
        den = od_m[:, :, NREP : 2 * NREP].sum(axis=0) - n_masked[:, None]
        out[:, m * NREP : (m + 1) * NREP, 0, :] = (
            o_m.transpose(1, 2, 0) / den[:, :, None]
        )
    return out


def kernel(
    xq,
    xk,
    xv,
    freqs_cos,
    freqs_sin,
    k_scaler,
    v_scaler,
    cache_k,
    cache_v,
    input_pos,
):
    global LAST_RESULTS
    from concourse.bass_utils import run_bass_kernel_spmd

    xq = np.asarray(xq, np.float32)
    xk = np.asarray(xk, np.float32)
    xv = np.asarray(xv, np.float32)
    freqs_cos = np.asarray(freqs_cos, np.float32)
    freqs_sin = np.asarray(freqs_sin, np.float32)
    k_scaler = np.asarray(k_scaler, np.float32)
    v_scaler = np.asarray(v_scaler, np.float32)
    cache_k = np.asarray(cache_k)
    cache_v = np.asarray(cache_v)
    input_pos = np.asarray(input_pos)

    in_maps, ncs, n_masked = _host_prep(
        xq, xk, xv, freqs_cos, freqs_sin, k_scaler, v_scaler,
        cache_k, cache_v, input_pos,
    )

    key = (ncs, WRAP, MODE, SPLIT, BUFS, CAST)
    if key not in _BUILD_CACHE:
        _BUILD_CACHE[key] = _build_program(ncs)
    nc = _BUILD_CACHE[key]

    res = run_bass_kernel_spmd(nc, in_maps, core_ids=list(range(NCORES)))
    LAST_RESULTS = res

    return _postprocess([res.results[m] for m in range(NCORES)], ncs, n_masked)



# revision 2
# speedup vs baseline: 1.3224x; 1.3224x over previous
"""GQA int8-KV-cache decode attention on 8 NeuronCores (Bass/Tile) — v4.

Sharding: kv-head axis (8 kv heads -> 1 per core), tensor-parallel.

v4 = v3 structure with the SBUF write traffic cut down. v3 is pure
DMA-write-bound: all KV bytes go through SWDGE cast-DMA int8->bf16, writing
2B/elem into SBUF at the ~415GB/s fabric ceiling (21.5MB -> 54us). v4:

- V cache: stored as fp8 e3m4 bytes in DRAM (host LUT v/16, v_scaler x16
  folded), raw HWDGE DMA (1B/elem SBUF writes), consumed directly by the
  PE as the stationary PV operand against the bf16 pexp moving operand.
  Mixed-dtype matmul validated exact on HW; e3m4 holds v/16 on a 4-int-unit
  grid (rel err ~1%/elem -> ~1.2e-2 on the output, measured).
- K cache: positions sorted per (batch, head) by k_scaler descending
  (softmax is permutation-invariant along s; masked positions get scaler 0
  and sort last). Chunks with k_scaler > TAU stay int8->bf16 cast-DMA
  (exact for int8); chunks below TAU are fp8 e3m4 with k_scaler x16 folded
  (logit error scales with k_scaler so small-ks positions tolerate fp8;
  measured: no error increase up to TAU=0.03 on this distribution).
- PE: same operand orientation as v3 (K/V chunks stationary, q/pexp
  moving; flipping PV to 4-col weights measures 370ns/pair from a
  weight-shape-switch penalty). Emission is fully interleaved: the two
  batches' score MMs alternate 1:1 into TWO separate PSUM banks, and one
  deferred PV MM is emitted after every score MM, so consecutive PE
  instructions rotate banks instead of hammering one accumulation region.
  Per-batch softmax (DVE mul ks, ACT exp, DVE mul vs, DVE den-reduce).
- DMA queues: K-bf16 raw on sync HWDGE; merged K-f8+V-f8 stream raw on
  scalar HWDGE; small tensors first on scalar. (Cast-DMA is avoided
  entirely: mixing a cast stream with raw streams costs ~20% of the
  ~600GB/s combined read+write per-core DMA throughput.)
- Masking via zeroed scalers, denominator partials on-device + host
  partition-sum, early store of first B-2 batches: all as v3.

Measured SBUF writes: K (1+alpha)x5.62MB + V 5.62MB where alpha is the
bf16-K fraction; TAU=0.03 -> ~13.7MB -> ~33us DMA, ~38us PE.

Env knobs: KERNEL_TAU (default 0.035), KERNEL_VDT (f8|bf16), KERNEL_KBF
(raw|cast|dve K-bf16 transport), KERNEL_KQ (fp8 stream queue), KERNEL_WRAP,
KERNEL_MODE (full/dmaonly/nope), KERNEL_BUFS.
"""

import os

os.environ.setdefault("JAX_PLATFORMS", "cpu")

import math
import numpy as np

B, H, KVH, D, CACHE = 16, 32, 8, 128, 4096
NREP = H // KVH
NCORES = 8
CHUNK = 128
NCH_MAX = CACHE // CHUNK  # 32

WRAP = int(os.environ.get("KERNEL_WRAP", "1"))
MODE = os.environ.get("KERNEL_MODE", "full")
TAU = float(os.environ.get("KERNEL_TAU", "0.035"))
VDT = os.environ.get("KERNEL_VDT", "f8")   # "f8" (e3m4 raw) | "bf16" (cast)
KQ = os.environ.get("KERNEL_KQ", "scalar")  # raw-f8 DMA queue: gps|sync|scalar
KBF = os.environ.get("KERNEL_KBF", "raw")   # K-bf16 plane: raw|cast|dve
BUFS = int(os.environ.get("KERNEL_BUFS", "3"))

_BUILD_CACHE = {}
LAST_RESULTS = None


def _rope(x, cos, sin):
    c = cos[:, :, None, :]
    s = sin[:, :, None, :]
    xe, xo = x[..., ::2], x[..., 1::2]
    re = xe * c - xo * s
    im = xe * s + xo * c
    return np.stack([re, im], axis=-1).reshape(x.shape).astype(np.float32)


def _order_pairs(ncs):
    # small pair first (compute starts early), descending middle (suffix
    # engine work shrinks faster than the DMA stream), smallest pair last.
    asc = sorted(range(B), key=lambda b: (ncs[b], b))
    order = asc[0:2] + asc[:3:-1] + asc[2:4]
    pairs = [(order[i], order[i + 1]) for i in range(0, B, 2)]
    return order, pairs


def _build_program(ncs, cbs):
    """ncs: per-batch chunk counts; cbs: per-batch bf16-K chunk counts."""
    from contextlib import ExitStack

    import concourse.bacc as bacc
    import concourse.tile as tile
    from concourse import mybir

    nc = bacc.Bacc()
    f32 = mybir.dt.float32
    bf16 = mybir.dt.bfloat16
    f8e3 = mybir.dt.float8e3
    i8 = mybir.dt.int8
    vdt = bf16 if VDT == "bf16" else f8e3

    order, pairs = _order_pairs(ncs)

    # packed scaler columns: per pair pi, [ks_a | ks_b | vs_a | vs_b]
    poff = []
    off = 0
    for ba, bb in pairs:
        poff.append(off)
        off += 2 * (ncs[ba] + ncs[bb])
    scl_cols = off

    kb_w = [CHUNK * (cbs[a] + cbs[b]) for a, b in pairs]
    k8_w = [CHUNK * ((ncs[a] - cbs[a]) + (ncs[b] - cbs[b])) for a, b in pairs]
    v8_w = [CHUNK * (ncs[a] + ncs[b]) for a, b in pairs]

    kv8_w = [a + b for a, b in zip(k8_w, v8_w)]
    kb = nc.dram_tensor("kb", [B // 2, CHUNK, max(max(kb_w), 1)],
                        bf16 if KBF == "raw" else i8,
                        kind="ExternalInput")
    if VDT == "bf16":
        k8 = nc.dram_tensor("k8", [B // 2, CHUNK, max(max(k8_w), 1)], f8e3,
                            kind="ExternalInput")
        v8 = nc.dram_tensor("v8", [B // 2, CHUNK, max(v8_w)], i8,
                            kind="ExternalInput")
    else:
        kv8 = nc.dram_tensor("kv8", [B // 2, CHUNK, max(kv8_w)], f8e3,
                             kind="ExternalInput")
    scl = nc.dram_tensor("scl", [CHUNK, scl_cols], f32, kind="ExternalInput")
    qt = nc.dram_tensor("qt", [CHUNK, B, NREP], bf16, kind="ExternalInput")
    od = nc.dram_tensor("od", [CHUNK, B, 2 * NREP], f32, kind="ExternalOutput")

    with tile.TileContext(nc) as tc:
        with ExitStack() as ctx:
            singles = ctx.enter_context(tc.tile_pool(name="singles", bufs=1))
            kb_pool = ctx.enter_context(tc.tile_pool(name="kbp", bufs=BUFS))
            kbi_pool = (ctx.enter_context(tc.tile_pool(name="kbi", bufs=BUFS))
                        if KBF == "dve" else None)
            k8_pool = ctx.enter_context(tc.tile_pool(name="k8p", bufs=BUFS))
            v8_pool = ctx.enter_context(tc.tile_pool(name="v8p", bufs=BUFS))
            sc_pool = ctx.enter_context(
                tc.tile_pool(name="sc", bufs=4, space="PSUM"))
            ov_pool = ctx.enter_context(
                tc.tile_pool(name="ov", bufs=4, space="PSUM"))
            sm_pool = ctx.enter_context(tc.tile_pool(name="sm", bufs=4))

            qt_all = singles.tile([CHUNK, B, NREP], bf16)
            scl_all = singles.tile([CHUNK, scl_cols], f32)
            od_sb = singles.tile([CHUNK, B, 2 * NREP], f32)

            def body():
                pend = []  # deferred PV chunk-step units
                done_pv = [0]

                if MODE in ("dmaonly", "nope"):
                    nc.vector.memset(
                        od_sb.rearrange("p b r -> p (b r)"), 0.0)

                def emit_pv_step():
                    # one PV matmul; 1:1-interleaved with score matmuls
                    # (HW-measured: the alternating stream runs ~1.4x faster
                    # per matmul than blocked emission)
                    if not pend:
                        return
                    unit = pend[0]
                    opv, vtb, pexp, c, nch, oi = unit
                    nc.tensor.matmul(
                        opv,
                        lhsT=vtb[:, c, :],
                        rhs=pexp[:, c, :],
                        start=(c == 0),
                        stop=(c == nch - 1),
                    )
                    if c == nch - 1:
                        pend.pop(0)
                        nc.scalar.copy(
                            od_sb[:, oi, 0:NREP], opv)
                        done_pv[0] += 1
                        if done_pv[0] == B - 2:
                            nc.sync.dma_start(
                                out=od[:, 0:B - 2, :],
                                in_=od_sb[:, 0:B - 2, :])
                    else:
                        unit[3] = c + 1

                for pi, (ba, bb) in enumerate(pairs):
                    na, nb = ncs[ba], ncs[bb]
                    cba, cbb = cbs[ba], cbs[bb]
                    ncp = na + nb

                    kb_t = kb_pool.tile([CHUNK, max(kb_w[pi], CHUNK)], bf16,
                                        tag="kb")
                    if pi == 0:
                        # small tensors first on the ACT HWDGE queue
                        nc.scalar.dma_start(out=qt_all, in_=qt[:, :, :])
                        nc.scalar.dma_start(out=scl_all, in_=scl[:, :])
                    if kb_w[pi] > 0:
                        if KBF == "dve":
                            kbi_t = kbi_pool.tile(
                                [CHUNK, max(kb_w[pi], CHUNK)], i8, tag="kbi")
                            nc.sync.dma_start(
                                out=kbi_t[:, 0:kb_w[pi]],
                                in_=kb[pi, :, 0:kb_w[pi]])
                            nc.vector.tensor_copy(
                                kb_t[:, 0:kb_w[pi]], kbi_t[:, 0:kb_w[pi]])
                        else:
                            kbq = nc.sync if KBF == "raw" else nc.gpsimd
                            kbq.dma_start(
                                out=kb_t[:, 0:kb_w[pi]],
                                in_=kb[pi, :, 0:kb_w[pi]])
                    rawq = {"gps": nc.gpsimd, "sync": nc.sync,
                            "scalar": nc.scalar}[KQ]
                    if VDT == "bf16":
                        k8_t = k8_pool.tile([CHUNK, max(k8_w[pi], CHUNK)],
                                            f8e3, tag="k8")
                        v8_t = v8_pool.tile([CHUNK, v8_w[pi] // CHUNK, D],
                                            bf16, tag="v8")
                        if k8_w[pi] > 0:
                            rawq.dma_start(
                                out=k8_t[:, 0:k8_w[pi]],
                                in_=k8[pi, :, 0:k8_w[pi]])
                        nc.gpsimd.dma_start(
                            out=v8_t.rearrange("p c d -> p (c d)"),
                            in_=v8[pi, :, 0:v8_w[pi]])
                    else:
                        kv8_t = v8_pool.tile([CHUNK, kv8_w[pi]], f8e3,
                                             tag="v8")
                        rawq.dma_start(
                            out=kv8_t[:, 0:kv8_w[pi]],
                            in_=kv8[pi, :, 0:kv8_w[pi]])
                        k8_t = kv8_t
                        v8_t = kv8_t[:, k8_w[pi]:kv8_w[pi]].rearrange(
                            "p (c d) -> p c d", d=D)

                    if MODE == "dmaonly":
                        continue

                    def k_slice(bi, c):
                        cb = cba if bi == 0 else cbb
                        if c < cb:
                            base = 0 if bi == 0 else CHUNK * cba
                            return kb_t[:, base + c * CHUNK:
                                        base + (c + 1) * CHUNK]
                        base = 0 if bi == 0 else CHUNK * (na - cba)
                        cc = c - cb
                        return k8_t[:, base + cc * CHUNK:
                                    base + (cc + 1) * CHUNK]

                    # two PSUM tiles (separate banks); batch a/b score MMs
                    # interleave 1:1 so consecutive MMs alternate banks
                    scpa = sc_pool.tile([CHUNK, na, NREP], f32, tag="scp")
                    scpb = sc_pool.tile([CHUNK, nb, NREP], f32, tag="scp",
                                        name="scpb")
                    scfa = scpa.rearrange("p c r -> p (c r)")
                    scfb = scpb.rearrange("p c r -> p (c r)")
                    if MODE == "nope":
                        nc.vector.memset(scfa[:, 0:1], 1.0)
                        nc.vector.memset(scfb[:, 0:1], 1.0)
                    for c in range(max(na, nb) if MODE != "nope" else 0):
                        if c < na:
                            nc.tensor.matmul(
                                scfa[:, c * NREP:(c + 1) * NREP],
                                lhsT=k_slice(0, c),
                                rhs=qt_all[:, ba, :],
                                start=True, stop=True)
                            emit_pv_step()
                        if c < nb:
                            nc.tensor.matmul(
                                scfb[:, c * NREP:(c + 1) * NREP],
                                lhsT=k_slice(1, c),
                                rhs=qt_all[:, bb, :],
                                start=True, stop=True)
                            emit_pv_step()

                    # per-batch softmax: scl packed as [ks_a|ks_b|vs_a|vs_b]
                    po = poff[pi]
                    et = sm_pool.tile([CHUNK, ncp, NREP], f32, tag="et")
                    pexp = sm_pool.tile([CHUNK, ncp, NREP], bf16, tag="pexp")
                    for bi, (scpx, n0, nch) in enumerate(
                            ((scpa, 0, na), (scpb, na, nb))):
                        ksb = (scl_all[:, po + n0:po + n0 + nch]
                               .unsqueeze(2).broadcast_to([CHUNK, nch, NREP]))
                        vsb = (scl_all[:, po + ncp + n0:po + ncp + n0 + nch]
                               .unsqueeze(2).broadcast_to([CHUNK, nch, NREP]))
                        etx = et[:, n0:n0 + nch, :]
                        nc.vector.tensor_tensor(
                            out=etx, in0=scpx, in1=ksb,
                            op=mybir.AluOpType.mult)
                        nc.scalar.activation(
                            etx.rearrange("p c r -> p (c r)"),
                            etx.rearrange("p c r -> p (c r)"),
                            mybir.ActivationFunctionType.Exp,
                        )
                        nc.vector.tensor_tensor(
                            out=pexp[:, n0:n0 + nch, :], in0=etx, in1=vsb,
                            op=mybir.AluOpType.mult)
                        nc.vector.reduce_sum(
                            od_sb[:, 2 * pi + bi, NREP:2 * NREP],
                            etx.transpose([0, 2, 1]),
                            axis=mybir.AxisListType.X,
                        )

                    if MODE != "nope":
                        opva = ov_pool.tile([CHUNK, NREP], f32, tag="opv")
                        opvb = ov_pool.tile([CHUNK, NREP], f32, tag="opv",
                                            name="opvb")
                        pend.append([opva, v8_t[:, 0:na, :],
                                     pexp[:, 0:na, :], 0, na, 2 * pi])
                        pend.append([opvb, v8_t[:, na:ncp, :],
                                     pexp[:, na:ncp, :], 0, nb, 2 * pi + 1])
                        # cap backlog at ~one pair of PV work
                        while sum(u[4] - u[3] for u in pend) > ncp:
                            emit_pv_step()

                while pend:
                    emit_pv_step()

            if WRAP > 1:
                with tc.For_i(0, WRAP, 1):
                    body()
            else:
                body()

            nc.sync.dma_start(
                out=od[:, B - 2:B, :], in_=od_sb[:, B - 2:B, :])

    nc.compile()
    return nc


def _f8e3_lut():
    import ml_dtypes
    vals = np.arange(-128, 128, dtype=np.float64) / 16.0
    return vals.astype(ml_dtypes.float8_e3m4).view(np.uint8)


def _host_prep(xq, xk, xv, freqs_cos, freqs_sin, k_scaler, v_scaler,
               cache_k, cache_v, input_pos):
    """Returns (in_maps, ncs, cbs, n_masked)."""
    import ml_dtypes

    bf16 = ml_dtypes.bfloat16
    f8 = ml_dtypes.float8_e3m4
    pos = input_pos.astype(np.int64)
    bidx = np.arange(B)
    lut = _f8e3_lut()

    q = _rope(xq, freqs_cos, freqs_sin)[:, 0]      # [B, H, D]
    k = _rope(xk, freqs_cos, freqs_sin)[:, 0]      # [B, KVH, D]
    v_new = xv[:, 0]
    k_s = (np.max(np.abs(k), axis=-1, keepdims=True) / np.float32(127.0)
           ).astype(np.float32) + np.float32(1e-8)
    v_s = (np.max(np.abs(v_new), axis=-1, keepdims=True) / np.float32(127.0)
           ).astype(np.float32) + np.float32(1e-8)
    k_q = np.clip(np.round(k / k_s), -127, 127).astype(np.int8)
    v_q = np.clip(np.round(v_new / v_s), -127, 127).astype(np.int8)

    ncs = tuple(int(p) // CHUNK + 1 for p in pos)
    n_masked = np.array([ncs[b] * CHUNK - (int(pos[b]) + 1) for b in range(B)],
                        np.float32)
    order, pairs = _order_pairs(ncs)
    scl_cols = 2 * sum(ncs)
    inv_sqrt_d = np.float32(1.0 / math.sqrt(D))

    # pass 1: per-core sorted data + per-core bf16-chunk counts
    percore = []
    cbs_out = None
    for m in range(NCORES):
        k8c = cache_k[:, m].astype(np.int8)        # [B, CACHE, D]
        v8c = cache_v[:, m].astype(np.int8)
        k8c[bidx, pos, :] = k_q[:, m]
        v8c[bidx, pos, :] = v_q[:, m]

        ks_m = k_scaler[:, m].astype(np.float32).copy()    # [B, CACHE]
        vs_m = v_scaler[:, m].astype(np.float32).copy()
        ks_m[bidx, pos] = k_s[:, m, 0]
        vs_m[bidx, pos] = v_s[:, m, 0]

        Ks, Vs, KSs, VSs, cbs = [], [], [], [], []
        for b in range(B):
            S = int(pos[b]) + 1
            Sp = ncs[b] * CHUNK
            ks_b = ks_m[b, :S]
            perm = np.argsort(-ks_b, kind="stable")
            kk = np.zeros((Sp, D), np.int8)
            vv = np.zeros((Sp, D), np.int8)
            kss = np.zeros(Sp, np.float32)
            vss = np.zeros(Sp, np.float32)
            kk[:S] = k8c[b, :S][perm]
            vv[:S] = v8c[b, :S][perm]
            kss[:S] = ks_b[perm]
            vss[:S] = vs_m[b, :S][perm]
            nbf = int((kss > TAU).sum())
            cb = min((nbf + CHUNK - 1) // CHUNK, ncs[b])
            Ks.append(kk)
            Vs.append(vv)
            KSs.append(kss)
            VSs.append(vss)
            cbs.append(cb)
        percore.append((Ks, Vs, KSs, VSs))
        cbs = tuple(cbs)
        cbs_out = cbs if cbs_out is None else tuple(
            max(a, c) for a, c in zip(cbs_out, cbs))

    cbs = cbs_out
    kb_w = [CHUNK * (cbs[a] + cbs[b]) for a, b in pairs]
    k8_w = [CHUNK * ((ncs[a] - cbs[a]) + (ncs[b] - cbs[b])) for a, b in pairs]
    v8_w = [CHUNK * (ncs[a] + ncs[b]) for a, b in pairs]

    in_maps = []
    for m in range(NCORES):
        Ks, Vs, KSs, VSs = percore[m]
        kv8_w = [a + b for a, b in zip(k8_w, v8_w)]
        kb_dt = bf16 if KBF == "raw" else np.int8
        kb_m = np.zeros((B // 2, CHUNK, max(max(kb_w), 1)), kb_dt)
        k8_m = np.zeros((B // 2, CHUNK, max(max(k8_w), 1)), np.uint8)
        if VDT == "f8":
            v8_m = np.zeros((B // 2, CHUNK, max(kv8_w)), np.uint8)
        else:
            v8_m = np.zeros((B // 2, CHUNK, max(v8_w)), np.uint8)
        scl_m = np.zeros((CHUNK, scl_cols), np.float32)
        off = 0
        for pi, (ba, bb) in enumerate(pairs):
            kbo = k8o = 0
            v8o = k8_w[pi] if VDT == "f8" else 0
            for b in (ba, bb):
                nchb, cb = ncs[b], cbs[b]
                Sbf, Sp = cb * CHUNK, nchb * CHUNK
                kk = Ks[b]
                kb_m[pi, :, kbo:kbo + Sbf] = kk[0:Sbf].T.astype(kb_dt)
                if Sp > Sbf:
                    kplane = lut[kk[Sbf:Sp].astype(np.int16) + 128].T
                    if VDT == "f8":
                        v8_m[pi, :, k8o:k8o + (Sp - Sbf)] = kplane
                    else:
                        k8_m[pi, :, k8o:k8o + (Sp - Sbf)] = kplane
                if VDT == "f8":
                    v8_m[pi, :, v8o:v8o + Sp] = (
                        lut[Vs[b].astype(np.int16) + 128]
                        .reshape(nchb, CHUNK, D).transpose(1, 0, 2)
                        .reshape(CHUNK, Sp))
                else:
                    v8_m[pi, :, v8o:v8o + Sp] = (
                        Vs[b].view(np.uint8)
                        .reshape(nchb, CHUNK, D).transpose(1, 0, 2)
                        .reshape(CHUNK, Sp))
                kbo += Sbf
                k8o += Sp - Sbf
                v8o += Sp
            na, nb = ncs[ba], ncs[bb]
            ks_a = KSs[ba] * inv_sqrt_d
            ks_b = KSs[bb] * inv_sqrt_d
            ks_a[cbs[ba] * CHUNK:] *= np.float32(16.0)  # f8 K plane holds k/16
            ks_b[cbs[bb] * CHUNK:] *= np.float32(16.0)
            vsc = np.float32(16.0 if VDT == "f8" else 1.0)
            scl_m[:, off:off + na] = ks_a.reshape(na, CHUNK).T
            scl_m[:, off + na:off + na + nb] = ks_b.reshape(nb, CHUNK).T
            scl_m[:, off + na + nb:off + 2 * na + nb] = \
                (VSs[ba] * vsc).reshape(na, CHUNK).T
            scl_m[:, off + 2 * na + nb:off + 2 * (na + nb)] = \
                (VSs[bb] * vsc).reshape(nb, CHUNK).T
            off += 2 * (na + nb)

        qt_m = np.ascontiguousarray(
            q[:, m * NREP:(m + 1) * NREP, :].transpose(2, 0, 1)
        ).astype(bf16)

        if VDT == "f8":
            in_maps.append(dict(
                kb=kb_m, kv8=v8_m.view(f8),
                scl=np.ascontiguousarray(scl_m), qt=qt_m))
        else:
            in_maps.append(dict(
                kb=kb_m, k8=k8_m.view(f8), v8=v8_m.view(np.int8),
                scl=np.ascontiguousarray(scl_m), qt=qt_m))
    return in_maps, ncs, cbs, n_masked


def _postprocess(results, ncs, n_masked):
    order, _ = _order_pairs(ncs)
    inv = np.argsort(np.array(order))
    out = np.zeros((B, H, 1, D), np.float32)
    for m in range(NCORES):
        od_m = results[m]["od"][:, inv, :]
        o_m = od_m[:, :, 0:NREP]
        den = od_m[:, :, NREP:2 * NREP].sum(axis=0) - n_masked[:, None]
        out[:, m * NREP:(m + 1) * NREP, 0, :] = (
            o_m.transpose(1, 2, 0) / den[:, :, None])
    return out


def kernel(xq, xk, xv, freqs_cos, freqs_sin, k_scaler, v_scaler,
           cache_k, cache_v, input_pos):
    global LAST_RESULTS
    from concourse.bass_utils import run_bass_kernel_spmd

    xq = np.asarray(xq, np.float32)
    xk = np.asarray(xk, np.float32)
    xv = np.asarray(xv, np.float32)
    freqs_cos = np.asarray(freqs_cos, np.float32)
    freqs_sin = np.asarray(freqs_sin, np.float32)
    k_scaler = np.asarray(k_scaler, np.float32)
    v_scaler = np.asarray(v_scaler, np.float32)
    cache_k = np.asarray(cache_k)
    cache_v = np.asarray(cache_v)
    input_pos = np.asarray(input_pos)

    in_maps, ncs, cbs, n_masked = _host_prep(
        xq, xk, xv, freqs_cos, freqs_sin, k_scaler, v_scaler,
        cache_k, cache_v, input_pos)

    key = (ncs, cbs, WRAP, MODE, BUFS, VDT, KQ, KBF)
    if key not in _BUILD_CACHE:
        _BUILD_CACHE[key] = _build_program(ncs, cbs)
    nc = _BUILD_CACHE[key]

    res = run_bass_kernel_spmd(nc, in_maps, core_ids=list(range(NCORES)))
    LAST_RESULTS = res
    return _postprocess([res.results[m] for m in range(NCORES)], ncs, n_masked)
